# revision 9
# baseline (speedup 1.0000x reference)
"""Trainium2 Bass kernel for nn_ComplexBlockLinear.

Math: per block n (8 blocks of 128 features), out = x @ W[n] with complex
x = x_re + i*x_im, W = wr + i*wi:
    out_re = xr @ wr - xi @ wi
    out_im = xr @ wi + xi @ wr

Strategy:
  - Data parallel: core b handles batch element b (B=8, 8 cores).
  - Host: transpose x[b] to [H, S] (feature-major) so the contraction dim
    lands on SBUF partitions, and split fp32 into bf16 hi+lo (Dekker split).
    fp32 matmul on TRN2 PE costs 4 cycles/col; bf16 costs 1, so the 3-term
    bf16 product (hi*hi + hi*lo + lo*hi) runs at 3/4 the fp32 cost with
    ~1e-5 relative error. Accumulation is fp32 in PSUM.
  - Device: weights stationary ([128i, 128o] per block), stream token
    chunks of 512; 12 matmuls per (block, chunk) accumulate psum_re/psum_im;
    DVE evacuates PSUM->SBUF; 1-2MB HWDGE DMAs both directions.
  - Host: transpose outputs back and interleave re/im.
"""

import os

import numpy as np
import ml_dtypes

B, S, H = 8, 4096, 1024
NBLK, BS = 8, 128
NCORES = 8
TCHUNK = 512
NCHUNK = S // TCHUNK

BF16 = ml_dtypes.bfloat16

# stationary weight pack order along the free axis
WRH, WRL, WIH, WIL, WIHN, WILN = range(6)

_CACHE = {}


def _build_module(nblk, s, tchunk):
    import concourse.mybir as mybir
    from concourse import bacc
    from concourse.tile import TileContext

    dt = mybir.dt
    h = nblk * BS
    nchunk = s // tchunk

    nc = bacc.Bacc(
        "TRN2",
        target_bir_lowering=False,
        debug=False,
        enable_asserts=False,
        num_devices=NCORES,
    )

    xrh = nc.dram_tensor("xrh", [h, s], dt.bfloat16, kind="ExternalInput").ap()
    xrl = nc.dram_tensor("xrl", [h, s], dt.bfloat16, kind="ExternalInput").ap()
    xih = nc.dram_tensor("xih", [h, s], dt.bfloat16, kind="ExternalInput").ap()
    xil = nc.dram_tensor("xil", [h, s], dt.bfloat16, kind="ExternalInput").ap()
    wpack = nc.dram_tensor(
        "wpack", [nblk, BS, 4 * BS], dt.bfloat16, kind="ExternalInput"
    ).ap()
    out_re = nc.dram_tensor("out_re", [h, s], dt.float32, kind="ExternalOutput").ap()
    out_im = nc.dram_tensor("out_im", [h, s], dt.float32, kind="ExternalOutput").ap()

    # feature-blocked DRAM views: [h, s] -> [p, n, t]
    xrh_v = xrh.rearrange("(n p) t -> p n t", p=BS)
    xrl_v = xrl.rearrange("(n p) t -> p n t", p=BS)
    xih_v = xih.rearrange("(n p) t -> p n t", p=BS)
    xil_v = xil.rearrange("(n p) t -> p n t", p=BS)
    ore_v = out_re.rearrange("(n p) t -> p n t", p=BS)
    oim_v = out_im.rearrange("(n p) t -> p n t", p=BS)
    w_v = wpack.rearrange("n i s -> i n s")

    with TileContext(nc) as tc:
        with (
            tc.tile_pool(name="wpool", bufs=1) as wpool,
            tc.tile_pool(name="xpool", bufs=4) as xpool,
            tc.tile_pool(name="opool", bufs=2) as opool,
            tc.tile_pool(name="psum", bufs=2, space="PSUM") as psum_pool,
        ):
            wt = wpool.tile([BS, nblk * 6 * BS], dt.bfloat16)
            wt_v = wt.rearrange("p (n s) -> p n s", s=6 * BS)

            def load_w(n, eng):
                # load wrh|wrl|wih|wil, then derive wihn|wiln = -wih|-wil
                eng.dma_start(out=wt_v[:, n, : 4 * BS], in_=w_v[:, n])
                nc.vector.tensor_scalar_mul(
                    wt_v[:, n, 4 * BS :], wt_v[:, n, 2 * BS : 4 * BS], -1.0
                )

            def wsl(n, k):
                return wt[:, (n * 6 + k) * BS : (n * 6 + k + 1) * BS]

            def load_x(c, head=False):
                # all loads on the sync ring; xrh/xrl first (first MMs need them)
                tsl = slice(c * tchunk, (c + 1) * tchunk)
                tiles = []
                for nm, view, eng in (
                    ("xrh_t", xrh_v, nc.sync),
                    ("xrl_t", xrl_v, nc.sync),
                    ("xih_t", xih_v, nc.sync),
                    ("xil_t", xil_v, nc.sync),
                ):
                    tile_ = xpool.tile([BS, nblk * tchunk], dt.bfloat16, name=nm)
                    eng.dma_start(
                        out=tile_.rearrange("p (n t) -> p n t", t=tchunk),
                        in_=view[:, :, tsl],
                    )
                    tiles.append(tile_)
                return tiles

            mm = nc.tensor.matmul

            def mm_xr(n, ps_re, ps_im, xr_h, xr_l, first):
                # the 6 terms sourced from x_re (need only xrh/xrl slabs)
                bsl = slice(n * tchunk, (n + 1) * tchunk)
                a, b = xr_h[:, bsl], xr_l[:, bsl]
                mm(ps_re, wsl(n, WRH), a, start=first, stop=False)
                mm(ps_re, wsl(n, WRH), b, start=False, stop=False)
                mm(ps_im, wsl(n, WIH), a, start=first, stop=False)
                mm(ps_im, wsl(n, WIH), b, start=False, stop=False)
                mm(ps_re, wsl(n, WRL), a, start=False, stop=False)
                mm(ps_im, wsl(n, WIL), a, start=False, stop=False)

            def mm_xi(n, ps_re, ps_im, xi_h, xi_l, first):
                # the 6 terms sourced from x_im
                bsl = slice(n * tchunk, (n + 1) * tchunk)
                a, b = xi_h[:, bsl], xi_l[:, bsl]
                mm(ps_re, wsl(n, WIHN), a, start=first, stop=False)
                mm(ps_re, wsl(n, WIHN), b, start=False, stop=False)
                mm(ps_im, wsl(n, WRH), a, start=first, stop=False)
                mm(ps_im, wsl(n, WRH), b, start=False, stop=False)
                mm(ps_re, wsl(n, WILN), a, start=False, stop=True)
                mm(ps_im, wsl(n, WRL), a, start=False, stop=True)

            def evac_half(c, half, pre, pim, fine=False):
                # copy 4 blocks' psums (2 two-bank tiles each) to SBUF and
                # store on the scalar HWDGE ring; re fully before im so the
                # re store launches ~2.4us earlier. fine=True (kernel tail)
                # stores per two-bank quarter to shorten the drain.
                tsl = slice(c * tchunk, (c + 1) * tchunk)
                osb_re = opool.tile([BS, 4 * tchunk], dt.float32, name="osb_re")
                osb_im = opool.tile([BS, 4 * tchunk], dt.float32, name="osb_im")
                for osb, ps, view in ((osb_re, pre, ore_v), (osb_im, pim, oim_v)):
                    for q in range(2):
                        qsl = slice(q * 2 * tchunk, (q + 1) * 2 * tchunk)
                        nc.vector.tensor_copy(osb[:, qsl], ps[q])
                        if fine:
                            nc.scalar.dma_start(
                                out=view[:, half * 4 + 2 * q : half * 4 + 2 * q + 2, tsl],
                                in_=osb[:, qsl].rearrange(
                                    "p (n t) -> p n t", t=tchunk
                                ),
                            )
                    if not fine:
                        nc.scalar.dma_start(
                            out=view[:, half * 4 : half * 4 + 4, tsl],
                            in_=osb.rearrange("p (n t) -> p n t", t=tchunk),
                        )

            def psum_pair():
                # two PSUM banks per tile; each matmul targets one bank slice
                ps_re = psum_pool.tile([BS, 2 * tchunk], dt.float32, name="ps_re")
                ps_im = psum_pool.tile([BS, 2 * tchunk], dt.float32, name="ps_im")
                return ps_re, ps_im

            def block_slices(ps_re, ps_im, k):
                ksl = slice(k * tchunk, (k + 1) * tchunk)
                return ps_re[:, ksl], ps_im[:, ksl]

            # ---- chunk 0: interleaved weight/x loads, xr-phase then xi-phase
            # over half-size block groups so PE starts as soon as w0+xrh land.
            load_w(0, nc.sync)
            xrh_t, xrl_t, xih_t, xil_t = load_x(0, head=True)
            for n in range(1, nblk):
                load_w(n, nc.sync)
            for half in range(2):
                pre = [None, None]
                pim = [None, None]
                for q in range(2):
                    pre[q], pim[q] = psum_pair()
                for n in range(half * 4, half * 4 + 4):
                    r, i_ = block_slices(pre[(n % 4) // 2], pim[(n % 4) // 2], n % 2)
                    mm_xr(n, r, i_, xrh_t, xrl_t, first=True)
                for n in range(half * 4, half * 4 + 4):
                    r, i_ = block_slices(pre[(n % 4) // 2], pim[(n % 4) // 2], n % 2)
                    mm_xi(n, r, i_, xih_t, xil_t, first=False)
                evac_half(0, half, pre, pim)

            # ---- steady chunks
            for c in range(1, nchunk):
                xrh_t, xrl_t, xih_t, xil_t = load_x(c)
                for half in range(2):
                    pre = [None, None]
                    pim = [None, None]
                    for q in range(2):
                        pre[q], pim[q] = psum_pair()
                        for k in range(2):
                            n = half * 4 + q * 2 + k
                            r, i_ = block_slices(pre[q], pim[q], k)
                            mm_xr(n, r, i_, xrh_t, xrl_t, first=True)
                            mm_xi(n, r, i_, xih_t, xil_t, first=False)
                    evac_half(
                        c, half, pre, pim,
                        fine=(c == nchunk - 1 and half == 1),
                    )

    nc.compile()
    return nc


def _get_module(nblk=NBLK, s=S, tchunk=TCHUNK):
    key = (nblk, s, tchunk)
    if key not in _CACHE:
        _CACHE[key] = _build_module(nblk, s, tchunk)
    return _CACHE[key]


def _split_bf16(x32):
    hi = x32.astype(BF16)
    lo = (x32 - hi.astype(np.float32)).astype(BF16)
    return np.ascontiguousarray(hi), np.ascontiguousarray(lo)


def _pack_weights(weight):
    wr = weight[..., 0].astype(np.float32)  # [n, i, o]
    wi = weight[..., 1].astype(np.float32)
    wrh = wr.astype(BF16)
    wrl = (wr - wrh.astype(np.float32)).astype(BF16)
    wih = wi.astype(BF16)
    wil = (wi - wih.astype(np.float32)).astype(BF16)
    return np.ascontiguousarray(np.concatenate([wrh, wrl, wih, wil], axis=2))


def _setup_trace_shim():
    """Make trace=True work under axon in containers lacking antenv.axon_hooks.

    Registers a stand-in antenv.axon_hooks module whose hook drives NTFF
    capture via ctypes on libaxon_pjrt.so (mirrors trn_agent_boot), and
    disables the S3 artifact upload in bass_utils.
    """
    import contextlib
    import ctypes
    import sys
    import types

    try:
        from antenv.axon_hooks import get_axon_ntff_profile_hook  # noqa: F401

        return
    except ImportError:
        pass

    so_path = "/opt/axon/libaxon_pjrt.so"
    lib = ctypes.CDLL(so_path)
    if not hasattr(lib, "axon_start_nrt_profile"):
        return
    lib.axon_start_nrt_profile.argtypes = [
        ctypes.POINTER(ctypes.c_int64),
        ctypes.c_size_t,
    ]
    lib.axon_start_nrt_profile.restype = ctypes.c_int64
    lib.axon_stop_nrt_profile.argtypes = [ctypes.c_char_p]
    lib.axon_stop_nrt_profile.restype = ctypes.c_int64

    @contextlib.contextmanager
    def _hook(output_dir, device_ids):
        import jax

        jax.devices()
        if device_ids:
            ids = (ctypes.c_int64 * len(device_ids))(*device_ids)
            rc = lib.axon_start_nrt_profile(ids, len(device_ids))
        else:
            rc = lib.axon_start_nrt_profile(None, 0)
        if rc != 0:
            raise RuntimeError(f"axon_start_nrt_profile rc={rc}")
        try:
            yield
        finally:
            n = lib.axon_stop_nrt_profile(str(output_dir).encode())
            print(f"ntff profile: {n} file(s) written to {output_dir}")

    mod = types.ModuleType("antenv.axon_hooks")
    mod.get_axon_ntff_profile_hook = lambda: _hook
    mod.set_axon_ntff_profile_hook = lambda h: None
    sys.modules["antenv.axon_hooks"] = mod

    from concourse import bass_utils

    bass_utils.upload_artifacts = lambda tmpdir: tmpdir


def kernel(x_re, x_im, weight):
    from concourse import bass_utils

    trace = bool(int(os.environ.get("KERNEL_TRACE", "0")))
    if trace:
        _setup_trace_shim()

    nc = _get_module()
    wpack = _pack_weights(weight)

    in_maps = []
    for b in range(NCORES):
        d = {"wpack": wpack}
        d["xrh"], d["xrl"] = _split_bf16(x_re[b].T.astype(np.float32))
        d["xih"], d["xil"] = _split_bf16(x_im[b].T.astype(np.float32))
        in_maps.append(d)

    res = bass_utils.run_bass_kernel_spmd(
        nc,
        in_maps,
        core_ids=list(range(NCORES)),
        trace=trace,
    )
    kernel._last_results = res

    out = np.empty((B, S, H, 2), np.float32)
    for b in range(NCORES):
        out[b, :, :, 0] = res.results[b]["out_re"].T
        out[b, :, :, 1] = res.results[b]["out_im"].T
    return out


kernel._last_results = None


# revision 10
# speedup vs baseline: 1.0428x; 1.0428x over previous
"""Trainium2 Bass kernel for nn_ComplexBlockLinear.

Math: per block n (8 blocks of 128 features), out = x @ W[n] with complex
x = x_re + i*x_im, W = wr + i*wi:
    out_re = xr @ wr - xi @ wi
    out_im = xr @ wi + xi @ wr

Strategy:
  - Data parallel: core b handles batch element b (B=8, 8 cores).
  - Host: transpose x[b] to [H, S] (feature-major) so the contraction dim
    lands on SBUF partitions, and split fp32 into bf16 hi+lo (Dekker split).
    fp32 matmul on TRN2 PE costs 4 cycles/col; bf16 costs 1, so the 3-term
    bf16 product (hi*hi + hi*lo + lo*hi) runs at 3/4 the fp32 cost with
    ~1e-5 relative error. Accumulation is fp32 in PSUM.
  - Device: weights stationary ([128i, 128o] per block), stream token
    chunks of 512; 12 matmuls per (block, chunk) accumulate psum_re/psum_im;
    DVE evacuates PSUM->SBUF; 1-2MB HWDGE DMAs both directions.
  - Host: transpose outputs back and interleave re/im.
"""

import os

import numpy as np
import ml_dtypes

B, S, H = 8, 4096, 1024
NBLK, BS = 8, 128
NCORES = 8
TCHUNK = 512
NCHUNK = S // TCHUNK

BF16 = ml_dtypes.bfloat16

# stationary weight pack order along the free axis
WRH, WRL, WIH, WIL, WIHN, WILN = range(6)

_CACHE = {}


def _build_module(nblk, s, tchunk):
    import concourse.mybir as mybir
    from concourse import bacc
    from concourse.tile import TileContext

    dt = mybir.dt
    h = nblk * BS
    nchunk = s // tchunk

    nc = bacc.Bacc(
        "TRN2",
        target_bir_lowering=False,
        debug=False,
        enable_asserts=False,
        num_devices=NCORES,
    )

    xrh = nc.dram_tensor("xrh", [h, s], dt.bfloat16, kind="ExternalInput").ap()
    xrl = nc.dram_tensor("xrl", [h, s], dt.bfloat16, kind="ExternalInput").ap()
    xih = nc.dram_tensor("xih", [h, s], dt.bfloat16, kind="ExternalInput").ap()
    xil = nc.dram_tensor("xil", [h, s], dt.bfloat16, kind="ExternalInput").ap()
    wpack = nc.dram_tensor(
        "wpack", [nblk, BS, 4 * BS], dt.bfloat16, kind="ExternalInput"
    ).ap()
    out_re = nc.dram_tensor("out_re", [h, s], dt.float32, kind="ExternalOutput").ap()
    out_im = nc.dram_tensor("out_im", [h, s], dt.float32, kind="ExternalOutput").ap()

    # feature-blocked DRAM views: [h, s] -> [p, n, t]
    xrh_v = xrh.rearrange("(n p) t -> p n t", p=BS)
    xrl_v = xrl.rearrange("(n p) t -> p n t", p=BS)
    xih_v = xih.rearrange("(n p) t -> p n t", p=BS)
    xil_v = xil.rearrange("(n p) t -> p n t", p=BS)
    ore_v = out_re.rearrange("(n p) t -> p n t", p=BS)
    oim_v = out_im.rearrange("(n p) t -> p n t", p=BS)
    w_v = wpack.rearrange("n i s -> i n s")

    with TileContext(nc) as tc:
        with (
            tc.tile_pool(name="wpool", bufs=1) as wpool,
            tc.tile_pool(name="xpool", bufs=4) as xpool,
            tc.tile_pool(name="opool", bufs=2) as opool,
            tc.tile_pool(name="psum", bufs=2, space="PSUM") as psum_pool,
        ):
            wt = wpool.tile([BS, nblk * 6 * BS], dt.bfloat16)
            wt_v = wt.rearrange("p (n s) -> p n s", s=6 * BS)

            def load_w(n, eng):
                # load wrh|wrl|wih|wil, then derive wihn|wiln = -wih|-wil
                eng.dma_start(out=wt_v[:, n, : 4 * BS], in_=w_v[:, n])
                nc.vector.tensor_scalar_mul(
                    wt_v[:, n, 4 * BS :], wt_v[:, n, 2 * BS : 4 * BS], -1.0
                )

            def wsl(n, k):
                return wt[:, (n * 6 + k) * BS : (n * 6 + k + 1) * BS]

            def load_x(c, head=False):
                # all loads on the sync ring; xrh/xrl first (first MMs need them)
                tsl = slice(c * tchunk, (c + 1) * tchunk)
                tiles = []
                for nm, view, eng in (
                    ("xrh_t", xrh_v, nc.sync),
                    ("xrl_t", xrl_v, nc.sync),
                    ("xih_t", xih_v, nc.sync),
                    ("xil_t", xil_v, nc.scalar),
                ):
                    tile_ = xpool.tile([BS, nblk * tchunk], dt.bfloat16, name=nm)
                    eng.dma_start(
                        out=tile_.rearrange("p (n t) -> p n t", t=tchunk),
                        in_=view[:, :, tsl],
                    )
                    tiles.append(tile_)
                return tiles

            mm = nc.tensor.matmul

            def mm_xr(n, ps_re, ps_im, xr_h, xr_l, first):
                # the 6 terms sourced from x_re (need only xrh/xrl slabs)
                bsl = slice(n * tchunk, (n + 1) * tchunk)
                a, b = xr_h[:, bsl], xr_l[:, bsl]
                mm(ps_re, wsl(n, WRH), a, start=first, stop=False)
                mm(ps_re, wsl(n, WRH), b, start=False, stop=False)
                mm(ps_im, wsl(n, WIH), a, start=first, stop=False)
                mm(ps_im, wsl(n, WIH), b, start=False, stop=False)
                mm(ps_re, wsl(n, WRL), a, start=False, stop=False)
                mm(ps_im, wsl(n, WIL), a, start=False, stop=False)

            def mm_xi(n, ps_re, ps_im, xi_h, xi_l, first):
                # the 6 terms sourced from x_im
                bsl = slice(n * tchunk, (n + 1) * tchunk)
                a, b = xi_h[:, bsl], xi_l[:, bsl]
                mm(ps_re, wsl(n, WIHN), a, start=first, stop=False)
                mm(ps_re, wsl(n, WIHN), b, start=False, stop=False)
                mm(ps_im, wsl(n, WRH), a, start=first, stop=False)
                mm(ps_im, wsl(n, WRH), b, start=False, stop=False)
                mm(ps_re, wsl(n, WILN), a, start=False, stop=True)
                mm(ps_im, wsl(n, WRL), a, start=False, stop=True)

            def evac_half(c, half, pre, pim, fine=False):
                # copy 4 blocks' psums (2 two-bank tiles each) to SBUF and
                # store on the scalar HWDGE ring; re fully before im so the
                # re store launches ~2.4us earlier. fine=True (kernel tail)
                # stores per two-bank quarter to shorten the drain.
                tsl = slice(c * tchunk, (c + 1) * tchunk)
                osb_re = opool.tile([BS, 4 * tchunk], dt.float32, name="osb_re")
                osb_im = opool.tile([BS, 4 * tchunk], dt.float32, name="osb_im")
                for osb, ps, view, eng in (
                    (osb_re, pre, ore_v, nc.scalar),
                    (osb_im, pim, oim_v, nc.gpsimd),
                ):
                    for q in range(2):
                        qsl = slice(q * 2 * tchunk, (q + 1) * 2 * tchunk)
                        nc.vector.tensor_copy(osb[:, qsl], ps[q])
                        if fine:
                            eng.dma_start(
                                out=view[:, half * 4 + 2 * q : half * 4 + 2 * q + 2, tsl],
                                in_=osb[:, qsl].rearrange(
                                    "p (n t) -> p n t", t=tchunk
                                ),
                            )
                    if not fine:
                        eng.dma_start(
                            out=view[:, half * 4 : half * 4 + 4, tsl],
                            in_=osb.rearrange("p (n t) -> p n t", t=tchunk),
                        )

            def psum_pair():
                # two PSUM banks per tile; each matmul targets one bank slice
                ps_re = psum_pool.tile([BS, 2 * tchunk], dt.float32, name="ps_re")
                ps_im = psum_pool.tile([BS, 2 * tchunk], dt.float32, name="ps_im")
                return ps_re, ps_im

            def block_slices(ps_re, ps_im, k):
                ksl = slice(k * tchunk, (k + 1) * tchunk)
                return ps_re[:, ksl], ps_im[:, ksl]

            # ---- chunk 0: interleaved weight/x loads, xr-phase then xi-phase
            # over half-size block groups so PE starts as soon as w0+xrh land.
            load_w(0, nc.sync)
            tsl0 = slice(0, tchunk)
            xrh_t = xpool.tile([BS, nblk * tchunk], dt.bfloat16, name="xrh_t")
            xrl_t = xpool.tile([BS, nblk * tchunk], dt.bfloat16, name="xrl_t")
            xih_t = xpool.tile([BS, nblk * tchunk], dt.bfloat16, name="xih_t")
            xil_t = xpool.tile([BS, nblk * tchunk], dt.bfloat16, name="xil_t")
            nc.scalar.dma_start(
                out=xil_t.rearrange("p (n t) -> p n t", t=tchunk),
                in_=xil_v[:, :, tsl0],
            )
            for tile_, view in ((xrh_t, xrh_v), (xrl_t, xrl_v)):
                nc.sync.dma_start(
                    out=tile_.rearrange("p (n t) -> p n t", t=tchunk),
                    in_=view[:, :, tsl0],
                )
            for n in (1, 2, 3):
                load_w(n, nc.sync)
            nc.sync.dma_start(
                out=xih_t.rearrange("p (n t) -> p n t", t=tchunk),
                in_=xih_v[:, :, tsl0],
            )
            for n in range(4, nblk):
                load_w(n, nc.sync)
            for half in range(2):
                pre = [None, None]
                pim = [None, None]
                for q in range(2):
                    pre[q], pim[q] = psum_pair()
                for n in range(half * 4, half * 4 + 4):
                    r, i_ = block_slices(pre[(n % 4) // 2], pim[(n % 4) // 2], n % 2)
                    mm_xr(n, r, i_, xrh_t, xrl_t, first=True)
                for n in range(half * 4, half * 4 + 4):
                    r, i_ = block_slices(pre[(n % 4) // 2], pim[(n % 4) // 2], n % 2)
                    mm_xi(n, r, i_, xih_t, xil_t, first=False)
                evac_half(0, half, pre, pim)

            # ---- steady chunks
            for c in range(1, nchunk):
                xrh_t, xrl_t, xih_t, xil_t = load_x(c)
                for half in range(2):
                    pre = [None, None]
                    pim = [None, None]
                    for q in range(2):
                        pre[q], pim[q] = psum_pair()
                        for k in range(2):
                            n = half * 4 + q * 2 + k
                            r, i_ = block_slices(pre[q], pim[q], k)
                            mm_xr(n, r, i_, xrh_t, xrl_t, first=True)
                            mm_xi(n, r, i_, xih_t, xil_t, first=False)
                    evac_half(
                        c, half, pre, pim,
                        fine=(c == nchunk - 1 and half == 1),
                    )

    nc.compile()
    return nc


def _get_module(nblk=NBLK, s=S, tchunk=TCHUNK):
    key = (nblk, s, tchunk)
    if key not in _CACHE:
        _CACHE[key] = _build_module(nblk, s, tchunk)
    return _CACHE[key]


def _split_bf16(x32):
    hi = x32.astype(BF16)
    lo = (x32 - hi.astype(np.float32)).astype(BF16)
    return np.ascontiguousarray(hi), np.ascontiguousarray(lo)


def _pack_weights(weight):
    wr = weight[..., 0].astype(np.float32)  # [n, i, o]
    wi = weight[..., 1].astype(np.float32)
    wrh = wr.astype(BF16)
    wrl = (wr - wrh.astype(np.float32)).astype(BF16)
    wih = wi.astype(BF16)
    wil = (wi - wih.astype(np.float32)).astype(BF16)
    return np.ascontiguousarray(np.concatenate([wrh, wrl, wih, wil], axis=2))


def _setup_trace_shim():
    """Make trace=True work under axon in containers lacking antenv.axon_hooks.

    Registers a stand-in antenv.axon_hooks module whose hook drives NTFF
    capture via ctypes on libaxon_pjrt.so (mirrors trn_agent_boot), and
    disables the S3 artifact upload in bass_utils.
    """
    import contextlib
    import ctypes
    import sys
    import types

    try:
        from antenv.axon_hooks import get_axon_ntff_profile_hook  # noqa: F401

        return
    except ImportError:
        pass

    so_path = "/opt/axon/libaxon_pjrt.so"
    lib = ctypes.CDLL(so_path)
    if not hasattr(lib, "axon_start_nrt_profile"):
        return
    lib.axon_start_nrt_profile.argtypes = [
        ctypes.POINTER(ctypes.c_int64),
        ctypes.c_size_t,
    ]
    lib.axon_start_nrt_profile.restype = ctypes.c_int64
    lib.axon_stop_nrt_profile.argtypes = [ctypes.c_char_p]
    lib.axon_stop_nrt_profile.restype = ctypes.c_int64

    @contextlib.contextmanager
    def _hook(output_dir, device_ids):
        import jax

        jax.devices()
        if device_ids:
            ids = (ctypes.c_int64 * len(device_ids))(*device_ids)
            rc = lib.axon_start_nrt_profile(ids, len(device_ids))
        else:
            rc = lib.axon_start_nrt_profile(None, 0)
        if rc != 0:
            raise RuntimeError(f"axon_start_nrt_profile rc={rc}")
        try:
            yield
        finally:
            n = lib.axon_stop_nrt_profile(str(output_dir).encode())
            print(f"ntff profile: {n} file(s) written to {output_dir}")

    mod = types.ModuleType("antenv.axon_hooks")
    mod.get_axon_ntff_profile_hook = lambda: _hook
    mod.set_axon_ntff_profile_hook = lambda h: None
    sys.modules["antenv.axon_hooks"] = mod

    from concourse import bass_utils

    bass_utils.upload_artifacts = lambda tmpdir: tmpdir


def kernel(x_re, x_im, weight):
    from concourse import bass_utils

    trace = bool(int(os.environ.get("KERNEL_TRACE", "0")))
    if trace:
        _setup_trace_shim()

    nc = _get_module()
    wpack = _pack_weights(weight)

    in_maps = []
    for b in range(NCORES):
        d = {"wpack": wpack}
        d["xrh"], d["xrl"] = _split_bf16(x_re[b].T.astype(np.float32))
        d["xih"], d["xil"] = _split_bf16(x_im[b].T.astype(np.float32))
        in_maps.append(d)

    res = bass_utils.run_bass_kernel_spmd(
        nc,
        in_maps,
        core_ids=list(range(NCORES)),
        trace=trace,
    )
    kernel._last_results = res

    out = np.empty((B, S, H, 2), np.float32)
    for b in range(NCORES):
        out[b, :, :, 0] = res.results[b]["out_re"].T
        out[b, :, :, 1] = res.results[b]["out_im"].T
    return out


kernel._last_results = None


# revision 11
# speedup vs baseline: 1.0605x; 1.0169x over previous
"""Trainium2 Bass kernel for nn_ComplexBlockLinear.

Math: per block n (8 blocks of 128 features), out = x @ W[n] with complex
x = x_re + i*x_im, W = wr + i*wi:
    out_re = xr @ wr - xi @ wi
    out_im = xr @ wi + xi @ wr

Strategy:
  - Data parallel: core b handles batch element b (B=8, 8 cores).
  - Host: transpose x[b] to [H, S] (feature-major) so the contraction dim
    lands on SBUF partitions, and split fp32 into bf16 hi+lo (Dekker split).
    fp32 matmul on TRN2 PE costs 4 cycles/col; bf16 costs 1, so the 3-term
    bf16 product (hi*hi + hi*lo + lo*hi) runs at 3/4 the fp32 cost with
    ~1e-5 relative error. Accumulation is fp32 in PSUM.
  - Device: weights stationary ([128i, 128o] per block), stream token
    chunks of 512; 12 matmuls per (block, chunk) accumulate psum_re/psum_im;
    DVE evacuates PSUM->SBUF; 1-2MB HWDGE DMAs both directions.
  - Host: transpose outputs back and interleave re/im.
"""

import os

import numpy as np
import ml_dtypes

B, S, H = 8, 4096, 1024
NBLK, BS = 8, 128
NCORES = 8
TCHUNK = 512
NCHUNK = S // TCHUNK

BF16 = ml_dtypes.bfloat16

# stationary weight pack order along the free axis
WRH, WRL, WIH, WIL, WIHN, WILN = range(6)

_CACHE = {}


def _build_module(nblk, s, tchunk):
    import concourse.mybir as mybir
    from concourse import bacc
    from concourse.tile import TileContext

    dt = mybir.dt
    h = nblk * BS
    nchunk = s // tchunk

    nc = bacc.Bacc(
        "TRN2",
        target_bir_lowering=False,
        debug=False,
        enable_asserts=False,
        num_devices=NCORES,
    )

    xrh = nc.dram_tensor("xrh", [h, s], dt.bfloat16, kind="ExternalInput").ap()
    xrl = nc.dram_tensor("xrl", [h, s], dt.bfloat16, kind="ExternalInput").ap()
    xih = nc.dram_tensor("xih", [h, s], dt.bfloat16, kind="ExternalInput").ap()
    xil = nc.dram_tensor("xil", [h, s], dt.bfloat16, kind="ExternalInput").ap()
    wpack = nc.dram_tensor(
        "wpack", [nblk, BS, 4 * BS], dt.bfloat16, kind="ExternalInput"
    ).ap()
    out_re = nc.dram_tensor("out_re", [h, s], dt.float32, kind="ExternalOutput").ap()
    out_im = nc.dram_tensor("out_im", [h, s], dt.float32, kind="ExternalOutput").ap()

    # feature-blocked DRAM views: [h, s] -> [p, n, t]
    xrh_v = xrh.rearrange("(n p) t -> p n t", p=BS)
    xrl_v = xrl.rearrange("(n p) t -> p n t", p=BS)
    xih_v = xih.rearrange("(n p) t -> p n t", p=BS)
    xil_v = xil.rearrange("(n p) t -> p n t", p=BS)
    ore_v = out_re.rearrange("(n p) t -> p n t", p=BS)
    oim_v = out_im.rearrange("(n p) t -> p n t", p=BS)
    w_v = wpack.rearrange("n i s -> i n s")

    with TileContext(nc) as tc:
        with (
            tc.tile_pool(name="wpool", bufs=1) as wpool,
            tc.tile_pool(name="xpool", bufs=4) as xpool,
            tc.tile_pool(name="opool", bufs=2) as opool,
            tc.tile_pool(name="psum", bufs=2, space="PSUM") as psum_pool,
        ):
            wt = wpool.tile([BS, nblk * 6 * BS], dt.bfloat16)
            wt_v = wt.rearrange("p (n s) -> p n s", s=6 * BS)

            def load_w(n, eng):
                # load wrh|wrl|wih|wil, then derive wihn|wiln = -wih|-wil
                eng.dma_start(out=wt_v[:, n, : 4 * BS], in_=w_v[:, n])
                nc.vector.tensor_scalar_mul(
                    wt_v[:, n, 4 * BS :], wt_v[:, n, 2 * BS : 4 * BS], -1.0
                )

            def wsl(n, k):
                return wt[:, (n * 6 + k) * BS : (n * 6 + k + 1) * BS]

            def load_x(c, xil_eng=None):
                # xrh/xrl/xih on the sync ring; xil on the scalar ring in
                # steady state (keeps the sync ring under its serialization
                # cap), on sync for the first chunks (protects the head
                # window from non-critical traffic).
                tsl = slice(c * tchunk, (c + 1) * tchunk)
                tiles = []
                for nm, view, eng in (
                    ("xrh_t", xrh_v, nc.sync),
                    ("xrl_t", xrl_v, nc.sync),
                    ("xih_t", xih_v, nc.sync),
                    ("xil_t", xil_v, xil_eng or nc.scalar),
                ):
                    tile_ = xpool.tile([BS, nblk * tchunk], dt.bfloat16, name=nm)
                    eng.dma_start(
                        out=tile_.rearrange("p (n t) -> p n t", t=tchunk),
                        in_=view[:, :, tsl],
                    )
                    tiles.append(tile_)
                return tiles

            mm = nc.tensor.matmul

            def mm_xr(n, ps_re, ps_im, xr_h, xr_l, first):
                # the 6 terms sourced from x_re (need only xrh/xrl slabs)
                bsl = slice(n * tchunk, (n + 1) * tchunk)
                a, b = xr_h[:, bsl], xr_l[:, bsl]
                mm(ps_re, wsl(n, WRH), a, start=first, stop=False)
                mm(ps_re, wsl(n, WRH), b, start=False, stop=False)
                mm(ps_im, wsl(n, WIH), a, start=first, stop=False)
                mm(ps_im, wsl(n, WIH), b, start=False, stop=False)
                mm(ps_re, wsl(n, WRL), a, start=False, stop=False)
                mm(ps_im, wsl(n, WIL), a, start=False, stop=False)

            def mm_xi(n, ps_re, ps_im, xi_h, xi_l, first):
                # the 6 terms sourced from x_im
                bsl = slice(n * tchunk, (n + 1) * tchunk)
                a, b = xi_h[:, bsl], xi_l[:, bsl]
                mm(ps_re, wsl(n, WIHN), a, start=first, stop=False)
                mm(ps_re, wsl(n, WIHN), b, start=False, stop=False)
                mm(ps_im, wsl(n, WRH), a, start=first, stop=False)
                mm(ps_im, wsl(n, WRH), b, start=False, stop=False)
                mm(ps_re, wsl(n, WILN), a, start=False, stop=True)
                mm(ps_im, wsl(n, WRL), a, start=False, stop=True)

            def evac_half(c, half, pre, pim, fine=False):
                # copy 4 blocks' psums (2 two-bank tiles each) to SBUF and
                # store on the scalar HWDGE ring; re fully before im so the
                # re store launches ~2.4us earlier. fine=True (kernel tail)
                # stores per two-bank quarter to shorten the drain.
                tsl = slice(c * tchunk, (c + 1) * tchunk)
                osb_re = opool.tile([BS, 4 * tchunk], dt.float32, name="osb_re")
                osb_im = opool.tile([BS, 4 * tchunk], dt.float32, name="osb_im")
                for osb, ps, view, eng in (
                    (osb_re, pre, ore_v, nc.scalar),
                    (osb_im, pim, oim_v, nc.gpsimd),
                ):
                    for q in range(2):
                        qsl = slice(q * 2 * tchunk, (q + 1) * 2 * tchunk)
                        nc.vector.tensor_copy(osb[:, qsl], ps[q])
                        if fine:
                            eng.dma_start(
                                out=view[:, half * 4 + 2 * q : half * 4 + 2 * q + 2, tsl],
                                in_=osb[:, qsl].rearrange(
                                    "p (n t) -> p n t", t=tchunk
                                ),
                            )
                    if not fine:
                        eng.dma_start(
                            out=view[:, half * 4 : half * 4 + 4, tsl],
                            in_=osb.rearrange("p (n t) -> p n t", t=tchunk),
                        )

            def psum_pair():
                # two PSUM banks per tile; each matmul targets one bank slice
                ps_re = psum_pool.tile([BS, 2 * tchunk], dt.float32, name="ps_re")
                ps_im = psum_pool.tile([BS, 2 * tchunk], dt.float32, name="ps_im")
                return ps_re, ps_im

            def block_slices(ps_re, ps_im, k):
                ksl = slice(k * tchunk, (k + 1) * tchunk)
                return ps_re[:, ksl], ps_im[:, ksl]

            # ---- chunk 0: interleaved weight/x loads, xr-phase then xi-phase
            # over half-size block groups so PE starts as soon as w0+xrh land.
            load_w(0, nc.sync)
            tsl0 = slice(0, tchunk)
            xrh_t = xpool.tile([BS, nblk * tchunk], dt.bfloat16, name="xrh_t")
            xrl_t = xpool.tile([BS, nblk * tchunk], dt.bfloat16, name="xrl_t")
            xih_t = xpool.tile([BS, nblk * tchunk], dt.bfloat16, name="xih_t")
            xil_t = xpool.tile([BS, nblk * tchunk], dt.bfloat16, name="xil_t")
            for tile_, view in ((xrh_t, xrh_v), (xrl_t, xrl_v)):
                nc.sync.dma_start(
                    out=tile_.rearrange("p (n t) -> p n t", t=tchunk),
                    in_=view[:, :, tsl0],
                )
            for n in (1, 2, 3):
                load_w(n, nc.sync)
            for tile_, view in ((xih_t, xih_v), (xil_t, xil_v)):
                nc.sync.dma_start(
                    out=tile_.rearrange("p (n t) -> p n t", t=tchunk),
                    in_=view[:, :, tsl0],
                )
            for n in range(4, nblk):
                load_w(n, nc.gpsimd)
            for half in range(2):
                pre = [None, None]
                pim = [None, None]
                for q in range(2):
                    pre[q], pim[q] = psum_pair()
                for n in range(half * 4, half * 4 + 4):
                    r, i_ = block_slices(pre[(n % 4) // 2], pim[(n % 4) // 2], n % 2)
                    mm_xr(n, r, i_, xrh_t, xrl_t, first=True)
                for n in range(half * 4, half * 4 + 4):
                    r, i_ = block_slices(pre[(n % 4) // 2], pim[(n % 4) // 2], n % 2)
                    mm_xi(n, r, i_, xih_t, xil_t, first=False)
                evac_half(0, half, pre, pim)

            # ---- steady chunks
            for c in range(1, nchunk):
                xrh_t, xrl_t, xih_t, xil_t = load_x(
                    c, xil_eng=nc.sync if c == 1 else None
                )
                for half in range(2):
                    pre = [None, None]
                    pim = [None, None]
                    for q in range(2):
                        pre[q], pim[q] = psum_pair()
                        for k in range(2):
                            n = half * 4 + q * 2 + k
                            r, i_ = block_slices(pre[q], pim[q], k)
                            mm_xr(n, r, i_, xrh_t, xrl_t, first=True)
                            mm_xi(n, r, i_, xih_t, xil_t, first=False)
                    evac_half(
                        c, half, pre, pim,
                        fine=(c == nchunk - 1 and half == 1),
                    )

    nc.compile()
    return nc


def _get_module(nblk=NBLK, s=S, tchunk=TCHUNK):
    key = (nblk, s, tchunk)
    if key not in _CACHE:
        _CACHE[key] = _build_module(nblk, s, tchunk)
    return _CACHE[key]


def _split_bf16(x32):
    hi = x32.astype(BF16)
    lo = (x32 - hi.astype(np.float32)).astype(BF16)
    return np.ascontiguousarray(hi), np.ascontiguousarray(lo)


def _pack_weights(weight):
    wr = weight[..., 0].astype(np.float32)  # [n, i, o]
    wi = weight[..., 1].astype(np.float32)
    wrh = wr.astype(BF16)
    wrl = (wr - wrh.astype(np.float32)).astype(BF16)
    wih = wi.astype(BF16)
    wil = (wi - wih.astype(np.float32)).astype(BF16)
    return np.ascontiguousarray(np.concatenate([wrh, wrl, wih, wil], axis=2))


def _setup_trace_shim():
    """Make trace=True work under axon in containers lacking antenv.axon_hooks.

    Registers a stand-in antenv.axon_hooks module whose hook drives NTFF
    capture via ctypes on libaxon_pjrt.so (mirrors trn_agent_boot), and
    disables the S3 artifact upload in bass_utils.
    """
    import contextlib
    import ctypes
    import sys
    import types

    try:
        from antenv.axon_hooks import get_axon_ntff_profile_hook  # noqa: F401

        return
    except ImportError:
        pass

    so_path = "/opt/axon/libaxon_pjrt.so"
    lib = ctypes.CDLL(so_path)
    if not hasattr(lib, "axon_start_nrt_profile"):
        return
    lib.axon_start_nrt_profile.argtypes = [
        ctypes.POINTER(ctypes.c_int64),
        ctypes.c_size_t,
    ]
    lib.axon_start_nrt_profile.restype = ctypes.c_int64
    lib.axon_stop_nrt_profile.argtypes = [ctypes.c_char_p]
    lib.axon_stop_nrt_profile.restype = ctypes.c_int64

    @contextlib.contextmanager
    def _hook(output_dir, device_ids):
        import jax

        jax.devices()
        if device_ids:
            ids = (ctypes.c_int64 * len(device_ids))(*device_ids)
            rc = lib.axon_start_nrt_profile(ids, len(device_ids))
        else:
            rc = lib.axon_start_nrt_profile(None, 0)
        if rc != 0:
            raise RuntimeError(f"axon_start_nrt_profile rc={rc}")
        try:
            yield
        finally:
            n = lib.axon_stop_nrt_profile(str(output_dir).encode())
            print(f"ntff profile: {n} file(s) written to {output_dir}")

    mod = types.ModuleType("antenv.axon_hooks")
    mod.get_axon_ntff_profile_hook = lambda: _hook
    mod.set_axon_ntff_profile_hook = lambda h: None
    sys.modules["antenv.axon_hooks"] = mod

    from concourse import bass_utils

    bass_utils.upload_artifacts = lambda tmpdir: tmpdir


def kernel(x_re, x_im, weight):
    from concourse import bass_utils

    trace = bool(int(os.environ.get("KERNEL_TRACE", "0")))
    if trace:
        _setup_trace_shim()

    nc = _get_module()
    wpack = _pack_weights(weight)

    in_maps = []
    for b in range(NCORES):
        d = {"wpack": wpack}
        d["xrh"], d["xrl"] = _split_bf16(x_re[b].T.astype(np.float32))
        d["xih"], d["xil"] = _split_bf16(x_im[b].T.astype(np.float32))
        in_maps.append(d)

    res = bass_utils.run_bass_kernel_spmd(
        nc,
        in_maps,
        core_ids=list(range(NCORES)),
        trace=trace,
    )
    kernel._last_results = res

    out = np.empty((B, S, H, 2), np.float32)
    for b in range(NCORES):
        out[b, :, :, 0] = res.results[b]["out_re"].T
        out[b, :, :, 1] = res.results[b]["out_im"].T
    return out


kernel._last_results = None


# revision 12
# speedup vs baseline: 1.0933x; 1.0309x over previous
"""Trainium2 Bass kernel for nn_ComplexBlockLinear.

Math: per block n (8 blocks of 128 features), out = x @ W[n] with complex
x = x_re + i*x_im, W = wr + i*wi:
    out_re = xr @ wr - xi @ wi
    out_im = xr @ wi + xi @ wr

Strategy:
  - Data parallel: core b handles batch element b (B=8, 8 cores).
  - Host: transpose x[b] to [H, S] (feature-major) so the contraction dim
    lands on SBUF partitions, and split fp32 into bf16 hi+lo (Dekker split).
    fp32 matmul on TRN2 PE costs 4 cycles/col; bf16 costs 1, so the 3-term
    bf16 product (hi*hi + hi*lo + lo*hi) runs at 3/4 the fp32 cost with
    ~1e-5 relative error. Accumulation is fp32 in PSUM.
  - Device: weights stationary ([128i, 128o] per block), stream token
    chunks of 512; 12 matmuls per (block, chunk) accumulate psum_re/psum_im;
    DVE evacuates PSUM->SBUF; 1-2MB HWDGE DMAs both directions.
  - Host: transpose outputs back and interleave re/im.
"""

import os

import numpy as np
import ml_dtypes

B, S, H = 8, 4096, 1024
NBLK, BS = 8, 128
NCORES = 8
TCHUNK = 512
NCHUNK = S // TCHUNK

BF16 = ml_dtypes.bfloat16

# stationary weight pack order along the free axis
WRH, WRL, WIH, WIL, WIHN, WILN = range(6)

_CACHE = {}


def _build_module(nblk, s, tchunk):
    import concourse.mybir as mybir
    from concourse import bacc
    from concourse.tile import TileContext

    dt = mybir.dt
    h = nblk * BS
    nchunk = s // tchunk

    nc = bacc.Bacc(
        "TRN2",
        target_bir_lowering=False,
        debug=False,
        enable_asserts=False,
        num_devices=NCORES,
    )

    xrh = nc.dram_tensor("xrh", [h, s], dt.bfloat16, kind="ExternalInput").ap()
    xrl = nc.dram_tensor("xrl", [h, s], dt.bfloat16, kind="ExternalInput").ap()
    xih = nc.dram_tensor("xih", [h, s], dt.bfloat16, kind="ExternalInput").ap()
    xil = nc.dram_tensor("xil", [h, s], dt.bfloat16, kind="ExternalInput").ap()
    wpack = nc.dram_tensor(
        "wpack", [nblk, BS, 4 * BS], dt.bfloat16, kind="ExternalInput"
    ).ap()
    out_re = nc.dram_tensor("out_re", [h, s], dt.float32, kind="ExternalOutput").ap()
    out_im = nc.dram_tensor("out_im", [h, s], dt.float32, kind="ExternalOutput").ap()

    # feature-blocked DRAM views: [h, s] -> [p, n, t]
    xrh_v = xrh.rearrange("(n p) t -> p n t", p=BS)
    xrl_v = xrl.rearrange("(n p) t -> p n t", p=BS)
    xih_v = xih.rearrange("(n p) t -> p n t", p=BS)
    xil_v = xil.rearrange("(n p) t -> p n t", p=BS)
    ore_v = out_re.rearrange("(n p) t -> p n t", p=BS)
    oim_v = out_im.rearrange("(n p) t -> p n t", p=BS)
    w_v = wpack.rearrange("n i s -> i n s")

    with TileContext(nc) as tc:
        with (
            tc.tile_pool(name="wpool", bufs=1) as wpool,
            tc.tile_pool(name="xpool", bufs=4) as xpool,
            tc.tile_pool(name="opool", bufs=3) as opool,
            tc.tile_pool(name="psum", bufs=2, space="PSUM") as psum_pool,
        ):
            wt = wpool.tile([BS, nblk * 6 * BS], dt.bfloat16)
            wt_v = wt.rearrange("p (n s) -> p n s", s=6 * BS)

            def load_w(n, eng):
                # load wrh|wrl|wih|wil, then derive wihn|wiln = -wih|-wil
                eng.dma_start(out=wt_v[:, n, : 4 * BS], in_=w_v[:, n])
                nc.vector.tensor_scalar_mul(
                    wt_v[:, n, 4 * BS :], wt_v[:, n, 2 * BS : 4 * BS], -1.0
                )

            def wsl(n, k):
                return wt[:, (n * 6 + k) * BS : (n * 6 + k + 1) * BS]

            def load_x(c, xil_eng=None):
                # xrh/xrl/xih on the sync ring; xil on the scalar ring in
                # steady state (keeps the sync ring under its serialization
                # cap), on sync for the first chunks (protects the head
                # window from non-critical traffic).
                tsl = slice(c * tchunk, (c + 1) * tchunk)
                tiles = []
                for nm, view, eng in (
                    ("xrh_t", xrh_v, nc.sync),
                    ("xrl_t", xrl_v, nc.sync),
                    ("xih_t", xih_v, nc.sync),
                    ("xil_t", xil_v, xil_eng or nc.scalar),
                ):
                    tile_ = xpool.tile([BS, nblk * tchunk], dt.bfloat16, name=nm)
                    eng.dma_start(
                        out=tile_.rearrange("p (n t) -> p n t", t=tchunk),
                        in_=view[:, :, tsl],
                    )
                    tiles.append(tile_)
                return tiles

            mm = nc.tensor.matmul

            def mm_xr(n, ps_re, ps_im, xr_h, xr_l, first):
                # the 6 terms sourced from x_re (need only xrh/xrl slabs)
                bsl = slice(n * tchunk, (n + 1) * tchunk)
                a, b = xr_h[:, bsl], xr_l[:, bsl]
                mm(ps_re, wsl(n, WRH), a, start=first, stop=False)
                mm(ps_re, wsl(n, WRH), b, start=False, stop=False)
                mm(ps_im, wsl(n, WIH), a, start=first, stop=False)
                mm(ps_im, wsl(n, WIH), b, start=False, stop=False)
                mm(ps_re, wsl(n, WRL), a, start=False, stop=False)
                mm(ps_im, wsl(n, WIL), a, start=False, stop=False)

            def mm_xi(n, ps_re, ps_im, xi_h, xi_l, first):
                # the 6 terms sourced from x_im
                bsl = slice(n * tchunk, (n + 1) * tchunk)
                a, b = xi_h[:, bsl], xi_l[:, bsl]
                mm(ps_re, wsl(n, WIHN), a, start=first, stop=False)
                mm(ps_re, wsl(n, WIHN), b, start=False, stop=False)
                mm(ps_im, wsl(n, WRH), a, start=first, stop=False)
                mm(ps_im, wsl(n, WRH), b, start=False, stop=False)
                mm(ps_re, wsl(n, WILN), a, start=False, stop=True)
                mm(ps_im, wsl(n, WRL), a, start=False, stop=True)

            def evac_half(c, half, pre, pim, fine=False):
                # copy 4 blocks' psums (2 two-bank tiles each) to SBUF and
                # store on the scalar HWDGE ring; re fully before im so the
                # re store launches ~2.4us earlier. fine=True (kernel tail)
                # stores per two-bank quarter to shorten the drain.
                tsl = slice(c * tchunk, (c + 1) * tchunk)
                osb_re = opool.tile([BS, 4 * tchunk], dt.float32, name="osb_re")
                osb_im = opool.tile([BS, 4 * tchunk], dt.float32, name="osb_im")
                for osb, ps, view, eng in (
                    (osb_re, pre, ore_v, nc.scalar),
                    (osb_im, pim, oim_v, nc.gpsimd),
                ):
                    for q in range(2):
                        qsl = slice(q * 2 * tchunk, (q + 1) * 2 * tchunk)
                        nc.vector.tensor_copy(osb[:, qsl], ps[q])
                        if fine:
                            eng.dma_start(
                                out=view[:, half * 4 + 2 * q : half * 4 + 2 * q + 2, tsl],
                                in_=osb[:, qsl].rearrange(
                                    "p (n t) -> p n t", t=tchunk
                                ),
                            )
                    if not fine:
                        eng.dma_start(
                            out=view[:, half * 4 : half * 4 + 4, tsl],
                            in_=osb.rearrange("p (n t) -> p n t", t=tchunk),
                        )

            def psum_pair():
                # two PSUM banks per tile; each matmul targets one bank slice
                ps_re = psum_pool.tile([BS, 2 * tchunk], dt.float32, name="ps_re")
                ps_im = psum_pool.tile([BS, 2 * tchunk], dt.float32, name="ps_im")
                return ps_re, ps_im

            def block_slices(ps_re, ps_im, k):
                ksl = slice(k * tchunk, (k + 1) * tchunk)
                return ps_re[:, ksl], ps_im[:, ksl]

            # ---- chunk 0: interleaved weight/x loads, xr-phase then xi-phase
            # over half-size block groups so PE starts as soon as w0+xrh land.
            tsl0 = slice(0, tchunk)
            xrh_t = xpool.tile([BS, nblk * tchunk], dt.bfloat16, name="xrh_t")
            xrl_t = xpool.tile([BS, nblk * tchunk], dt.bfloat16, name="xrl_t")
            xih_t = xpool.tile([BS, nblk * tchunk], dt.bfloat16, name="xih_t")
            xil_t = xpool.tile([BS, nblk * tchunk], dt.bfloat16, name="xil_t")
            for tile_, view in ((xrh_t, xrh_v), (xrl_t, xrl_v)):
                nc.sync.dma_start(
                    out=tile_.rearrange("p (n t) -> p n t", t=tchunk),
                    in_=view[:, :, tsl0],
                )
            for n in (0, 1, 2, 3):
                load_w(n, nc.scalar)
            for tile_, view in ((xih_t, xih_v), (xil_t, xil_v)):
                nc.sync.dma_start(
                    out=tile_.rearrange("p (n t) -> p n t", t=tchunk),
                    in_=view[:, :, tsl0],
                )
            for n in range(4, nblk):
                load_w(n, nc.gpsimd)
            for half in range(2):
                pre = [None, None]
                pim = [None, None]
                for q in range(2):
                    pre[q], pim[q] = psum_pair()
                for n in range(half * 4, half * 4 + 4):
                    r, i_ = block_slices(pre[(n % 4) // 2], pim[(n % 4) // 2], n % 2)
                    mm_xr(n, r, i_, xrh_t, xrl_t, first=True)
                for n in range(half * 4, half * 4 + 4):
                    r, i_ = block_slices(pre[(n % 4) // 2], pim[(n % 4) // 2], n % 2)
                    mm_xi(n, r, i_, xih_t, xil_t, first=False)
                evac_half(0, half, pre, pim)

            # ---- steady chunks
            for c in range(1, nchunk):
                xrh_t, xrl_t, xih_t, xil_t = load_x(
                    c, xil_eng=nc.sync if c == 1 else None
                )
                for half in range(2):
                    pre = [None, None]
                    pim = [None, None]
                    for q in range(2):
                        pre[q], pim[q] = psum_pair()
                        for k in range(2):
                            n = half * 4 + q * 2 + k
                            r, i_ = block_slices(pre[q], pim[q], k)
                            mm_xr(n, r, i_, xrh_t, xrl_t, first=True)
                            mm_xi(n, r, i_, xih_t, xil_t, first=False)
                    evac_half(
                        c, half, pre, pim,
                        fine=(c == nchunk - 1 and half == 1),
                    )

    nc.compile()
    return nc


def _get_module(nblk=NBLK, s=S, tchunk=TCHUNK):
    key = (nblk, s, tchunk)
    if key not in _CACHE:
        _CACHE[key] = _build_module(nblk, s, tchunk)
    return _CACHE[key]


def _split_bf16(x32):
    hi = x32.astype(BF16)
    lo = (x32 - hi.astype(np.float32)).astype(BF16)
    return np.ascontiguousarray(hi), np.ascontiguousarray(lo)


def _pack_weights(weight):
    wr = weight[..., 0].astype(np.float32)  # [n, i, o]
    wi = weight[..., 1].astype(np.float32)
    wrh = wr.astype(BF16)
    wrl = (wr - wrh.astype(np.float32)).astype(BF16)
    wih = wi.astype(BF16)
    wil = (wi - wih.astype(np.float32)).astype(BF16)
    return np.ascontiguousarray(np.concatenate([wrh, wrl, wih, wil], axis=2))


def _setup_trace_shim():
    """Make trace=True work under axon in containers lacking antenv.axon_hooks.

    Registers a stand-in antenv.axon_hooks module whose hook drives NTFF
    capture via ctypes on libaxon_pjrt.so (mirrors trn_agent_boot), and
    disables the S3 artifact upload in bass_utils.
    """
    import contextlib
    import ctypes
    import sys
    import types

    try:
        from antenv.axon_hooks import get_axon_ntff_profile_hook  # noqa: F401

        return
    except ImportError:
        pass

    so_path = "/opt/axon/libaxon_pjrt.so"
    lib = ctypes.CDLL(so_path)
    if not hasattr(lib, "axon_start_nrt_profile"):
        return
    lib.axon_start_nrt_profile.argtypes = [
        ctypes.POINTER(ctypes.c_int64),
        ctypes.c_size_t,
    ]
    lib.axon_start_nrt_profile.restype = ctypes.c_int64
    lib.axon_stop_nrt_profile.argtypes = [ctypes.c_char_p]
    lib.axon_stop_nrt_profile.restype = ctypes.c_int64

    @contextlib.contextmanager
    def _hook(output_dir, device_ids):
        import jax

        jax.devices()
        if device_ids:
            ids = (ctypes.c_int64 * len(device_ids))(*device_ids)
            rc = lib.axon_start_nrt_profile(ids, len(device_ids))
        else:
            rc = lib.axon_start_nrt_profile(None, 0)
        if rc != 0:
            raise RuntimeError(f"axon_start_nrt_profile rc={rc}")
        try:
            yield
        finally:
            n = lib.axon_stop_nrt_profile(str(output_dir).encode())
            print(f"ntff profile: {n} file(s) written to {output_dir}")

    mod = types.ModuleType("antenv.axon_hooks")
    mod.get_axon_ntff_profile_hook = lambda: _hook
    mod.set_axon_ntff_profile_hook = lambda h: None
    sys.modules["antenv.axon_hooks"] = mod

    from concourse import bass_utils

    bass_utils.upload_artifacts = lambda tmpdir: tmpdir


def kernel(x_re, x_im, weight):
    from concourse import bass_utils

    trace = bool(int(os.environ.get("KERNEL_TRACE", "0")))
    if trace:
        _setup_trace_shim()

    nc = _get_module()
    wpack = _pack_weights(weight)

    in_maps = []
    for b in range(NCORES):
        d = {"wpack": wpack}
        d["xrh"], d["xrl"] = _split_bf16(x_re[b].T.astype(np.float32))
        d["xih"], d["xil"] = _split_bf16(x_im[b].T.astype(np.float32))
        in_maps.append(d)

    res = bass_utils.run_bass_kernel_spmd(
        nc,
        in_maps,
        core_ids=list(range(NCORES)),
        trace=trace,
    )
    kernel._last_results = res

    out = np.empty((B, S, H, 2), np.float32)
    for b in range(NCORES):
        out[b, :, :, 0] = res.results[b]["out_re"].T
        out[b, :, :, 1] = res.results[b]["out_im"].T
    return out


kernel._last_results = None
